# revision 6
# baseline (speedup 1.0000x reference)
"""GCN layer on 8 TRN2 cores — hybrid DMA-gather + GPSIMD ap_gather lanes.

Sharding: each core owns 6250 contiguous dst nodes (edges grouped by dst on
host). Two gather lanes per core, split by src range:

  Lane P (Pool/GPSIMD): edges with src < 20480. A feature-major fp32 table
    [96, 10241] lives in SBUF (ch 0:48 = feats of nodes [0,10240), ch 48:96 =
    [10240,20480), col 10240 = zero row). ap_gather pulls per-slot columns
    (slots = rank-planes per dst-lane, Z-padded); Act converts to bf16; a
    constant fold matrix [96, 48] contracts channels on TensorE, accumulating
    the block aggregate transposed in PSUM; planes sum via PSUM accumulation.

  Lane D (DMA/SWDGE): remaining edges via the baseline pair-table gather
    (256B rows, int16 idx) with W=12 main slots per (node, parity) plus
    lvl1/lvl2 one-hot overflow matmuls.

Per block both lanes' partial aggregates add, then cj-scale, zero-in-degree
fallback blend, transpose, replicated 48x48 linear + relu.
"""

import numpy as np

N = 50000
E = 1600000
D = 48
NCORES = 8
NPC = 6250             # nodes per core
BLOCKS = 49            # node range padded to 49*128 = 6272
NPAD = BLOCKS * 128
PAIRS = 25088          # rows in the pair table (incl. zero rows)
ZPAIR = 25000          # an all-zero pair row used for padding slots
WMAIN = 12             # main slots per (node, parity), DMA lane
GROUPS = 4             # 32-node groups per block
OVG_T = 1              # level-1 overflow tiles per group

WINDOW = 20480         # pool lane takes src < WINDOW
HALF = WINDOW // 2
NE2 = HALF + 1         # pool table elems per channel half (incl zero col)
CHCAP = 96             # max sum of W2 per ap_gather chunk (x128 slots <= 12288)

_CACHE = {}


def _host_prep(features, src, dst):
    src = np.asarray(src).astype(np.int64)
    dst = np.asarray(dst).astype(np.int64)
    feats = np.asarray(features, dtype=np.float32)

    out_deg = np.bincount(src, minlength=N).astype(np.int32)
    in_deg = np.bincount(dst, minlength=N).astype(np.int32)

    # ---------------- pool lane (src < WINDOW) ---------------------------
    poolm = src < WINDOW
    srcP, dstP = src[poolm], dst[poolm]
    wdeg = np.bincount(dstP, minlength=N).astype(np.int64)

    sortP = np.argsort(dstP, kind="stable")
    ksP = dstP[sortP]
    rsP = np.r_[0, np.flatnonzero(np.diff(ksP)) + 1]
    ridP = np.zeros(len(ksP), np.int64)
    ridP[rsP] = 1
    ridP = np.cumsum(ridP) - 1
    rankP = np.empty(len(ksP), np.int64)
    rankP[sortP] = np.arange(len(ksP)) - rsP[ridP]

    wd = np.zeros((NCORES, NPAD), np.int64)
    for c in range(NCORES):
        wd[c, :NPC] = wdeg[c * NPC:c * NPC + NPC]
    W2 = wd.reshape(NCORES, BLOCKS, 128).max(axis=(0, 2))
    W2 = np.maximum(W2 + (W2 % 2), 2)            # even, >=2
    starts = np.r_[0, np.cumsum(W2 * 128)]
    TOTS = int(starts[-1])

    coreP = dstP // NPC
    nlP = dstP - coreP * NPC
    blockP = nlP // 128
    vP = nlP % 128
    slotP = starts[blockP] + rankP * 128 + vP

    lov = np.where(srcP < HALF, srcP, HALF).astype(np.int16)
    hiv = np.where(srcP >= HALF, srcP - HALF, HALF).astype(np.int16)

    # chunks of whole blocks, sum W2 <= CHCAP, each >= table for efficiency
    chunks = []
    b0 = 0
    while b0 < BLOCKS:
        b1, tot = b0, 0
        while b1 < BLOCKS and tot + W2[b1] <= CHCAP:
            tot += W2[b1]
            b1 += 1
        chunks.append((b0, b1, int(starts[b0]), int(tot * 128)))
        b0 = b1
    CH = tuple(chunks)

    pidx_pc = []
    for c in range(NCORES):
        plo = np.full(TOTS, HALF, np.int16)
        phi = np.full(TOTS, HALF, np.int16)
        sel = coreP == c
        plo[slotP[sel]] = lov[sel]
        phi[slotP[sel]] = hiv[sel]
        pid = np.zeros((96, TOTS // 16), np.int16)
        for (cb0, cb1, s0, n) in CH:
            wlo = plo[s0:s0 + n].reshape(n // 16, 16).T
            whi = phi[s0:s0 + n].reshape(n // 16, 16).T
            c0 = s0 // 16
            for g in range(3):
                pid[g * 16:(g + 1) * 16, c0:c0 + n // 16] = wlo
                pid[48 + g * 16:48 + (g + 1) * 16, c0:c0 + n // 16] = whi
        pidx_pc.append(np.ascontiguousarray(pid))

    ptab = np.zeros((96, NE2), np.float32)
    ptab[0:48, 0:HALF] = feats[0:HALF].T
    ptab[48:96, 0:HALF] = feats[HALF:WINDOW].T
    pdeg = np.ones((96, NE2), np.int16)
    dg = np.maximum(out_deg, 1)
    pdeg[0:48, 0:HALF] = dg[0:HALF][None, :]
    pdeg[48:96, 0:HALF] = dg[HALF:WINDOW][None, :]

    # ---------------- DMA lane (src >= WINDOW) ---------------------------
    dsel = ~poolm
    srcd, dstd = src[dsel], dst[dsel]

    par = (srcd & 1).astype(np.int64)
    pair = (srcd >> 1).astype(np.int64)

    fpair = np.zeros((PAIRS, 96), np.float32)
    fpair[: N // 2, 0:48] = feats[0::2]
    fpair[: N // 2, 48:96] = feats[1::2]

    def pairwise_deg(deg_half):
        full = np.concatenate([deg_half, np.zeros(PAIRS - N // 2, np.int32)])
        return np.ascontiguousarray(full.reshape(PAIRS // 128, 128).T)

    degE_dev = pairwise_deg(out_deg[0::2])
    degO_dev = pairwise_deg(out_deg[1::2])

    key = dstd * 2 + par
    sort2 = np.argsort(key, kind="stable")
    ks = key[sort2]
    runstart = np.r_[0, np.flatnonzero(np.diff(ks)) + 1]
    runid = np.zeros(len(ks), np.int64)
    runid[runstart] = 1
    runid = np.cumsum(runid) - 1
    rank = np.empty(len(ks), np.int64)
    rank[sort2] = np.arange(len(ks)) - runstart[runid]

    core = dstd // NPC
    nl = dstd - core * NPC
    block = nl // 128
    v = nl % 128
    g = v // 32

    selm = rank < WMAIN
    Tm = par[selm] * WMAIN + g[selm] * (WMAIN // 4) + rank[selm] // 4
    lanem = (v[selm] % 32) * 4 + rank[selm] % 4

    selo = ~selm
    okey = (core[selo] * BLOCKS + block[selo]) * GROUPS + g[selo]
    osort = np.argsort(okey, kind="stable")
    oks = okey[osort]
    orunstart = np.r_[0, np.flatnonzero(np.diff(oks)) + 1]
    orunid = np.zeros(len(oks), np.int64)
    orunid[orunstart] = 1
    orunid = np.cumsum(orunid) - 1
    q = np.empty(len(oks), np.int64)
    q[osort] = np.arange(len(oks)) - orunstart[orunid]

    lvl1 = q < OVG_T * 128
    sel2 = ~lvl1
    oidx = np.flatnonzero(selo)
    e2 = oidx[sel2]
    k2 = core[e2] * BLOCKS + block[e2]
    s2 = np.argsort(k2, kind="stable")
    k2s = k2[s2]
    if len(k2s):
        rs2 = np.r_[0, np.flatnonzero(np.diff(k2s)) + 1]
        rid2 = np.zeros(len(k2s), np.int64)
        rid2[rs2] = 1
        rid2 = np.cumsum(rid2) - 1
        q2 = np.empty(len(k2s), np.int64)
        q2[s2] = np.arange(len(k2s)) - rs2[rid2]
        OV2_T = max(1, int(np.ceil((q2.max() + 1) / 128)))
    else:
        q2 = np.zeros(0, np.int64)
        OV2_T = 1
    assert OV2_T <= 4, f"unexpectedly deep level-2 overflow: {OV2_T}"

    TILES = 2 * WMAIN + GROUPS * OVG_T + OV2_T
    NIDX = TILES * 128

    gidx = np.full((NCORES, BLOCKS, TILES, 128), ZPAIR, np.int32)
    mg = np.zeros((NCORES, BLOCKS, GROUPS * OVG_T, 2, 128, 32), np.float16)
    m2 = np.zeros((NCORES, BLOCKS, OV2_T, 2, 128, 128), np.float16)

    gidx[core[selm], block[selm], Tm, lanem] = pair[selm]

    e1 = oidx[lvl1]
    t1 = q[lvl1] // 128
    lane1 = q[lvl1] % 128
    T1 = 2 * WMAIN + g[e1] * OVG_T + t1
    gidx[core[e1], block[e1], T1, lane1] = pair[e1]
    mg[core[e1], block[e1], g[e1] * OVG_T + t1, par[e1], lane1, v[e1] % 32] = 1.0

    t2 = q2 // 128
    lane2 = q2 % 128
    T2 = 2 * WMAIN + GROUPS * OVG_T + t2
    gidx[core[e2], block[e2], T2, lane2] = pair[e2]
    m2[core[e2], block[e2], t2, par[e2], lane2, v[e2]] = 1.0

    base_cnt = (2 * WMAIN + GROUPS * OVG_T) * 128
    ov2_cnt = np.zeros((NCORES, BLOCKS), np.int64)
    np.add.at(ov2_cnt, (core[e2], block[e2]), 1)
    counts = base_cnt + np.where(np.arange(BLOCKS)[None, :] < 5,
                                 OV2_T * 128, ov2_cnt)
    ar = np.arange(128)
    for t in range(OV2_T):
        pad = (ar[None, None, :] >=
               np.clip(ov2_cnt - t * 128, 0, 128)[:, :, None])
        pad = pad & (np.arange(BLOCKS)[None, :, None] >= 5)
        gidx[:, :, 2 * WMAIN + GROUPS * OVG_T + t, :][pad] = -1

    per_core = []
    for c in range(NCORES):
        flat = gidx[c].reshape(BLOCKS, NIDX).astype(np.int16)
        wrapped = flat.reshape(BLOCKS, NIDX // 16, 16).transpose(0, 2, 1)
        gidx_w = np.broadcast_to(
            wrapped[:, None, :, :], (BLOCKS, 8, 16, NIDX // 16)
        ).reshape(BLOCKS, 128, NIDX // 16).copy()

        mg_dev = np.ascontiguousarray(
            mg[c].transpose(0, 3, 1, 2, 4).reshape(
                BLOCKS, 128, GROUPS * OVG_T * 2 * 32))
        m2_dev = np.ascontiguousarray(
            m2[c].transpose(0, 3, 1, 2, 4).reshape(
                BLOCKS, 128, OV2_T * 2 * 128))

        nlo = c * NPC
        ind = np.zeros(NPAD, np.int32)
        ind[:NPC] = in_deg[nlo:nlo + NPC]
        indeg_dev = np.ascontiguousarray(ind.reshape(BLOCKS, 128).T)

        fc = np.zeros((NPAD, D), np.float32)
        fc[:NPC] = feats[nlo:nlo + NPC]
        featc_dev = np.ascontiguousarray(
            fc.reshape(BLOCKS, 128, D).transpose(1, 0, 2))

        blk = np.concatenate([
            gidx_w.view(np.uint8).reshape(BLOCKS, 128, -1),
            mg_dev.view(np.uint8).reshape(BLOCKS, 128, -1),
            m2_dev.view(np.uint8).reshape(BLOCKS, 128, -1),
        ], axis=2)
        per_core.append(dict(blk=np.ascontiguousarray(blk),
                             counts=np.ascontiguousarray(
                                 counts[c].astype(np.int32)[None, :]),
                             indeg=indeg_dev, featc=featc_dev,
                             pidx=pidx_pc[c]))

    meta = dict(OV2_T=OV2_T, TILES=TILES, NIDX=NIDX,
                W2=tuple(int(x) for x in W2), CH=CH, TOTS=TOTS)
    return fpair, degE_dev, degO_dev, ptab, pdeg, per_core, meta


def _build_program(meta):
    import concourse.tile as tile
    from concourse import bacc, mybir

    OV2_T, TILES, NIDX = meta["OV2_T"], meta["TILES"], meta["NIDX"]
    W2, CH, TOTS = meta["W2"], meta["CH"], meta["TOTS"]
    starts = [0]
    for w in W2:
        starts.append(starts[-1] + w * 128)
    f16 = mybir.dt.float16
    bf16 = mybir.dt.bfloat16
    f32 = mybir.dt.float32
    i32 = mybir.dt.int32
    i16 = mybir.dt.int16
    AF = mybir.ActivationFunctionType
    OP = mybir.AluOpType

    nc = bacc.Bacc("TRN2", target_bir_lowering=False, debug=False,
                   num_devices=NCORES, num_swdge_queues=2)

    fpair = nc.dram_tensor("fpair", [PAIRS, 96], f32, kind="ExternalInput").ap()
    degE = nc.dram_tensor("degE", [128, PAIRS // 128], i32, kind="ExternalInput").ap()
    degO = nc.dram_tensor("degO", [128, PAIRS // 128], i32, kind="ExternalInput").ap()
    GIB = (NIDX // 16) * 2
    MGB = GROUPS * OVG_T * 2 * 32 * 2
    M2B = OV2_T * 2 * 128 * 2
    BLKB = GIB + MGB + M2B
    u8 = mybir.dt.uint8
    blkD = nc.dram_tensor("blk", [BLOCKS, 128, BLKB], u8, kind="ExternalInput").ap()
    cntD = nc.dram_tensor("cnt", [1, BLOCKS], i32, kind="ExternalInput").ap()
    indegD = nc.dram_tensor("indeg", [128, BLOCKS], i32, kind="ExternalInput").ap()
    featcD = nc.dram_tensor("featc", [128, BLOCKS, D], f32, kind="ExternalInput").ap()
    wbD = nc.dram_tensor("wb", [D + 1, D], f32, kind="ExternalInput").ap()
    identD = nc.dram_tensor("ident", [128, 128], f32, kind="ExternalInput").ap()
    id32D = nc.dram_tensor("id32", [128, GROUPS * 32], f16, kind="ExternalInput").ap()
    ptabD = nc.dram_tensor("ptab", [96, NE2], f32, kind="ExternalInput").ap()
    pdegD = nc.dram_tensor("pdeg", [96, NE2], i16, kind="ExternalInput").ap()
    pidxD = nc.dram_tensor("pidx", [96, TOTS // 16], i16, kind="ExternalInput").ap()
    foldD = nc.dram_tensor("foldm", [96, D], f32, kind="ExternalInput").ap()
    outD = nc.dram_tensor("out", [D, NPAD], f32, kind="ExternalOutput").ap()

    xtab = nc.dram_tensor("xtab", [PAIRS, 128], f16).ap()
    CP = PAIRS // 128  # 196 pair-columns

    with tile.TileContext(nc) as tc:
        with tc.tile_pool(name="const", bufs=1) as cpool, \
             tc.tile_pool(name="big", bufs=1) as bigpool:

            wb_s = cpool.tile([D + 1, D], f32, tag="wb")
            nc.sync.dma_start(out=wb_s[:], in_=wbD)
            wb_b = cpool.tile([D + 1, D], bf16, tag="wbb")
            nc.vector.tensor_copy(wb_b[:], wb_s[:])
            ident = cpool.tile([128, 128], f32, tag="ident")
            nc.sync.dma_start(out=ident[:], in_=identD)
            id32 = cpool.tile([128, GROUPS * 32], f16, tag="id32")
            nc.sync.dma_start(out=id32[:], in_=id32D)
            foldf = cpool.tile([96, D], f32, tag="foldf")
            nc.sync.dma_start(out=foldf[:], in_=foldD)
            foldb = cpool.tile([96, D], bf16, tag="foldb")
            nc.vector.tensor_copy(foldb[:], foldf[:])

            # ---- pool lane table: load + ci scale ------------------------
            ptab_s = bigpool.tile([96, NE2], f32, tag="ptab")
            nc.sync.dma_start(out=ptab_s[:], in_=ptabD)

            # ---- phase 1: ci per parity + fp16 scaled pair table ---------
            ciE = cpool.tile([128, CP], f32, tag="ciE")
            ciO = cpool.tile([128, CP], f32, tag="ciO")
            with tc.tile_pool(name="xb", bufs=2) as xbpool:
                for deg_ap, ci in ((degE, ciE), (degO, ciO)):
                    dint = xbpool.tile([128, CP], i32, tag="dint")
                    nc.sync.dma_start(out=dint[:], in_=deg_ap)
                    nc.vector.tensor_copy(ci[:], dint[:])
                    nc.vector.tensor_scalar_max(ci[:], ci[:], 1.0)
                    nc.scalar.activation(ci[:], ci[:], AF.Sqrt)
                    nc.vector.reciprocal(ci[:], ci[:])

                XC = 49
                for cc in range(CP // XC):
                    sl = slice(cc * XC, (cc + 1) * XC)
                    fin = xbpool.tile([128, XC, 96], f32, tag="fin")
                    nc.sync.dma_start(
                        out=fin[:],
                        in_=fpair.rearrange("(c p) d -> p c d", p=128)[:, sl, :])
                    xt = xbpool.tile([128, XC, 128], f16, tag="xt")
                    nc.vector.tensor_tensor(
                        xt[:, :, 0:48], fin[:, :, 0:48],
                        ciE[:, sl].unsqueeze(2).to_broadcast([128, XC, 48]),
                        OP.mult)
                    nc.vector.tensor_tensor(
                        xt[:, :, 64:112], fin[:, :, 48:96],
                        ciO[:, sl].unsqueeze(2).to_broadcast([128, XC, 48]),
                        OP.mult)
                    nc.sync.dma_start(
                        out=xtab.rearrange("(c p) d -> p c d", p=128)[:, sl, :],
                        in_=xt[:])

            # pool table scaling: ptab *= rsqrt(clamped outdeg)
            with tc.tile_pool(name="cif", bufs=1) as cifpool:
                cif = cifpool.tile([96, NE2], f32, tag="cif")
                pdg = cifpool.tile([96, NE2], i16, tag="pdg")
                nc.sync.dma_start(out=pdg[:], in_=pdegD)
                nc.vector.tensor_copy(cif[:], pdg[:])
                nc.scalar.activation(cif[:], cif[:], AF.Sqrt)
                nc.vector.reciprocal(cif[:], cif[:])
                nc.vector.tensor_mul(ptab_s[:], ptab_s[:], cif[:])

            # ---- per-node scaling/blend coefficients ---------------------
            indeg_f = cpool.tile([128, BLOCKS], f32, tag="indegf")
            with tc.tile_pool(name="tmp0", bufs=1) as t0pool:
                indeg_i = t0pool.tile([128, BLOCKS], i32, tag="indegi")
                nc.sync.dma_start(out=indeg_i[:], in_=indegD)
                nc.vector.tensor_copy(indeg_f[:], indeg_i[:])
            mask = cpool.tile([128, BLOCKS], f32, tag="mask")
            nc.vector.tensor_scalar(mask[:], indeg_f[:], 0.0, None, OP.is_gt)
            cjm = cpool.tile([128, BLOCKS], f32, tag="cjm")
            nc.vector.tensor_scalar_max(cjm[:], indeg_f[:], 1.0)
            nc.scalar.activation(cjm[:], cjm[:], AF.Sqrt)
            nc.vector.reciprocal(cjm[:], cjm[:])
            nc.vector.tensor_mul(cjm[:], cjm[:], mask[:])
            im1 = cpool.tile([128, BLOCKS], f32, tag="im1")
            nc.vector.tensor_scalar(im1[:], mask[:], -1.0, 1.0,
                                    OP.mult, OP.add)

            featc_s = bigpool.tile([128, BLOCKS, D], f32, tag="featc")
            nc.sync.dma_start(out=featc_s[:], in_=featcD)
            fb_s = bigpool.tile([128, BLOCKS, D], f32, tag="fb")
            nc.vector.tensor_tensor(
                fb_s[:], featc_s[:],
                im1[:, :].unsqueeze(2).to_broadcast([128, BLOCKS, D]),
                OP.mult)
            hT1 = bigpool.tile([D + 1, BLOCKS * 128], bf16, tag="hT1")
            nc.vector.memset(hT1[:, :], 1.0)

            # ---- phase 2 ------------------------------------------------
            cnt_s = cpool.tile([1, BLOCKS], i32, tag="cnt")
            nc.sync.dma_start(out=cnt_s[:], in_=cntD)
            nidx_reg_a = nc.gpsimd.alloc_register("nidx_a")
            nidx_reg_b = nc.gpsimd.alloc_register("nidx_b")
            nidx_regs = [nidx_reg_a, nidx_reg_b]
            with tc.tile_pool(name="blk", bufs=3) as blkpool, \
                 tc.tile_pool(name="msg", bufs=3) as msgpool, \
                 tc.tile_pool(name="pox", bufs=1) as poxpool, \
                 tc.tile_pool(name="pix", bufs=2) as pixpool, \
                 tc.tile_pool(name="pob", bufs=2) as pobpool, \
                 tc.tile_pool(name="sm", bufs=4) as smpool, \
                 tc.tile_pool(name="ps", bufs=2, space="PSUM") as pspool, \
                 tc.tile_pool(name="pp", bufs=2, space="PSUM") as pppool, \
                 tc.tile_pool(name="aux", bufs=1, space="PSUM") as auxpool:

                pout = poxpool.tile([96, CHCAP * 128], f32, tag="pout")

                for (cb0, cb1, s0, nch) in CH:
                    pidxt = pixpool.tile([96, CHCAP * 8], i16, tag="pidxt")
                    nc.sync.dma_start(
                        out=pidxt[:, 0:nch // 16],
                        in_=pidxD[:, s0 // 16:(s0 + nch) // 16])
                    nc.gpsimd.ap_gather(
                        out_ap=pout[:, 0:nch].unsqueeze(2),
                        in_ap=ptab_s[:].unsqueeze(2),
                        idxs_ap=pidxt[:, 0:nch // 16],
                        channels=96, num_elems=NE2, d=1, num_idxs=nch)

                    for b in range(cb0, cb1):
                        w2 = W2[b]
                        loff = starts[b] - s0
                        nb = w2 * 128

                        # pool partial: bf16 convert + fold-mm accumulate
                        poutb = pobpool.tile([96, CHCAP * 128 // 3], bf16,
                                             tag="poutb")
                        nc.scalar.activation(poutb[:, 0:nb],
                                             pout[:, loff:loff + nb],
                                             AF.Identity)
                        ps2 = pppool.tile([48, 256], f32, tag="ps2")
                        nmm = nb // 256
                        for j in range(nmm):
                            nc.tensor.matmul(
                                ps2[:, :], lhsT=foldb[:],
                                rhs=poutb[:, j * 256:(j + 1) * 256],
                                start=(j == 0), stop=(j == nmm - 1),
                                skip_group_check=True)
                        s2 = smpool.tile([48, 256], f32, tag="s2")
                        nc.vector.tensor_copy(s2[:], ps2[:])
                        pagg = smpool.tile([48, 128], f32, tag="pagg")
                        nc.vector.tensor_add(pagg[:], s2[:, 0:128],
                                             s2[:, 128:256])
                        ps3 = pspool.tile([128, 48], f32, tag="tp2")
                        nc.tensor.transpose(ps3[:], pagg[:],
                                            ident[0:48, 0:48])
                        pu = smpool.tile([128, 48], f32, tag="pu")
                        nc.vector.tensor_copy(pu[:], ps3[:])

                        # ---- DMA lane -----------------------------------
                        blkt = blkpool.tile([128, BLKB], u8, tag="blkt")
                        nc.sync.dma_start(out=blkt[:], in_=blkD[b])
                        gi = blkt[:, 0:GIB].bitcast(i16)
                        mgt = blkt[:, GIB:GIB + MGB].bitcast(f16)
                        m2t = blkt[:, GIB + MGB:BLKB].bitcast(f16)

                        msg = msgpool.tile([128, TILES, 128], f16, tag="msg")
                        nc.gpsimd.reg_load(nidx_regs[b % 2], cnt_s[0:1, b:b + 1])
                        nc.gpsimd.dma_gather(
                            out_ap=msg[:],
                            in_ap=xtab,
                            idxs_ap=gi,
                            num_idxs=NIDX,
                            num_idxs_reg=nidx_regs[b % 2],
                            elem_size=128,
                            queue_num=b % 2,
                            single_packet=False,
                        )

                        ps = pspool.tile([128, D], f32, tag="ps")
                        JW = WMAIN // 4
                        for p, c0 in ((0, 0), (1, 64)):
                            for gg in range(GROUPS):
                                for j in range(JW):
                                    T = p * WMAIN + gg * JW + j
                                    nc.tensor.matmul(
                                        ps[32 * gg:32 * (gg + 1), :],
                                        lhsT=id32[:, 32 * gg:32 * (gg + 1)],
                                        rhs=msg[:, T, c0:c0 + D],
                                        start=(p == 0 and j == 0),
                                        stop=False, skip_group_check=True,
                                        tile_position=(0, 32 * gg))
                        for gg in range(GROUPS):
                            for t in range(OVG_T):
                                T = 2 * WMAIN + gg * OVG_T + t
                                base = (gg * OVG_T + t) * 64
                                for p, c0 in ((0, 0), (1, 64)):
                                    nc.tensor.matmul(
                                        ps[32 * gg:32 * (gg + 1), :],
                                        lhsT=mgt[:, base + p * 32:base + p * 32 + 32],
                                        rhs=msg[:, T, c0:c0 + D],
                                        start=False, stop=False,
                                        skip_group_check=True,
                                        tile_position=(0, 32 * gg))
                        for t in range(OV2_T):
                            T = 2 * WMAIN + GROUPS * OVG_T + t
                            base = t * 256
                            for pi, (p, c0) in enumerate(((0, 0), (1, 64))):
                                last = (t == OV2_T - 1) and (pi == 1)
                                nc.tensor.matmul(
                                    ps[:, :],
                                    lhsT=m2t[:, base + p * 128:base + (p + 1) * 128],
                                    rhs=msg[:, T, c0:c0 + D],
                                    start=False, stop=last,
                                    skip_group_check=True,
                                    tile_position=(0, 0))

                        # combine lanes, cj-scale, blend, transpose
                        sagg = smpool.tile([128, D], f32, tag="sagg")
                        nc.vector.tensor_add(sagg[:], ps[:], pu[:])
                        t0 = smpool.tile([128, D], f32, tag="t0")
                        nc.vector.tensor_tensor(
                            t0[:], sagg[:],
                            cjm[:, b:b + 1].to_broadcast([128, D]), OP.mult)
                        hb = smpool.tile([128, D], f32, tag="hb")
                        nc.vector.tensor_add(hb[:], t0[:], fb_s[:, b, :])

                        tp = auxpool.tile([D, 128], f32, tag="aux")
                        nc.tensor.transpose(tp[:], hb[:], ident[:])
                        nc.vector.tensor_copy(hT1[0:D, b * 128:(b + 1) * 128],
                                              tp[:])

                # ---- linear + relu (transposed) --------------------------
                CHN = 512
                outT = bigpool.tile([D, BLOCKS * 128], f32, tag="outT")
                nch2 = (BLOCKS * 128 + CHN - 1) // CHN
                for i in range(nch2):
                    lo = i * CHN
                    hi = min(lo + CHN, BLOCKS * 128)
                    po = auxpool.tile([D, CHN], f32, tag="po")
                    nc.tensor.matmul(po[:, 0:hi - lo], lhsT=wb_b[:],
                                     rhs=hT1[:, lo:hi], start=True, stop=True)
                    nc.scalar.activation(outT[:, lo:hi], po[:, 0:hi - lo],
                                         AF.Relu)

                nc.sync.dma_start(out=outD, in_=outT[:])

    nc.compile()
    return nc


def kernel(features, src, dst, W, b):
    from concourse.bass_utils import run_bass_kernel_spmd

    fpair, degE, degO, ptab, pdeg, per_core, meta = _host_prep(
        features, src, dst)

    key = (meta["OV2_T"], meta["W2"], meta["CH"])
    if key not in _CACHE:
        _CACHE[key] = _build_program(meta)
    nc = _CACHE[key]

    Wb = np.concatenate([np.asarray(W, np.float32).T,
                         np.asarray(b, np.float32)[None, :]], axis=0)
    ident = np.eye(128, dtype=np.float32)
    id32 = np.zeros((128, GROUPS * 32), np.float16)
    lanes = np.arange(128)
    for gg in range(GROUPS):
        id32[lanes, gg * 32 + lanes // 4] = 1.0
    foldm = np.zeros((96, D), np.float32)
    for c in range(D):
        foldm[c, c] = 1.0
        foldm[48 + c, c] = 1.0

    in_maps = []
    for c in range(NCORES):
        pc = per_core[c]
        in_maps.append({
            "fpair": fpair, "degE": degE, "degO": degO,
            "blk": pc["blk"], "cnt": pc["counts"],
            "indeg": pc["indeg"], "featc": pc["featc"],
            "wb": Wb, "ident": ident, "id32": id32,
            "ptab": ptab, "pdeg": pdeg, "pidx": pc["pidx"],
            "foldm": foldm,
        })

    res = run_bass_kernel_spmd(nc, in_maps, core_ids=list(range(NCORES)))
    globals()["LAST_RESULTS"] = res
    out = np.concatenate(
        [res.results[c]["out"][:, :NPC].T for c in range(NCORES)], axis=0)
    return np.ascontiguousarray(out, dtype=np.float32)


# revision 7
# speedup vs baseline: 1.0946x; 1.0946x over previous
"""GCN layer on 8 TRN2 cores — hybrid DMA-gather + GPSIMD ap_gather lanes.

Sharding: each core owns 6250 contiguous dst nodes (edges grouped by dst on
host). Two gather lanes per core, split by src range:

  Lane P (Pool/GPSIMD): edges with src < 20480. A feature-major fp32 table
    [96, 10241] lives in SBUF (ch 0:48 = feats of nodes [0,10240), ch 48:96 =
    [10240,20480), col 10240 = zero row). ap_gather pulls per-slot columns
    (slots = rank-planes per dst-lane, Z-padded); Act converts to bf16; a
    constant fold matrix [96, 48] contracts channels on TensorE, accumulating
    the block aggregate transposed in PSUM; planes sum via PSUM accumulation.

  Lane D (DMA/SWDGE): remaining edges via the baseline pair-table gather
    (256B rows, int16 idx) with W=12 main slots per (node, parity) plus
    lvl1/lvl2 one-hot overflow matmuls.

Per block both lanes' partial aggregates add, then cj-scale, zero-in-degree
fallback blend, transpose, replicated 48x48 linear + relu.
"""

import numpy as np

N = 50000
E = 1600000
D = 48
NCORES = 8
NPC = 6250             # nodes per core
BLOCKS = 49            # node range padded to 49*128 = 6272
NPAD = BLOCKS * 128
PAIRS = 25088          # rows in the pair table (incl. zero rows)
ZPAIR = 25000          # an all-zero pair row used for padding slots
WMAIN = 12             # main slots per (node, parity), DMA lane
GROUPS = 4             # 32-node groups per block
OVG_T = 1              # level-1 overflow tiles per group

WINDOW = 20480         # pool lane takes src < WINDOW
HALF = WINDOW // 2
NE2 = HALF + 1         # pool table elems per channel half (incl zero col)
CHCAP = 96             # max sum of W2 per ap_gather chunk (x128 slots <= 12288)

_CACHE = {}


def _host_prep(features, src, dst):
    src = np.asarray(src).astype(np.int64)
    dst = np.asarray(dst).astype(np.int64)
    feats = np.asarray(features, dtype=np.float32)

    out_deg = np.bincount(src, minlength=N).astype(np.int32)
    in_deg = np.bincount(dst, minlength=N).astype(np.int32)

    # ---------------- pool lane (src < WINDOW) ---------------------------
    poolm = src < WINDOW
    srcP, dstP = src[poolm], dst[poolm]
    wdeg = np.bincount(dstP, minlength=N).astype(np.int64)

    sortP = np.argsort(dstP, kind="stable")
    ksP = dstP[sortP]
    rsP = np.r_[0, np.flatnonzero(np.diff(ksP)) + 1]
    ridP = np.zeros(len(ksP), np.int64)
    ridP[rsP] = 1
    ridP = np.cumsum(ridP) - 1
    rankP = np.empty(len(ksP), np.int64)
    rankP[sortP] = np.arange(len(ksP)) - rsP[ridP]

    wd = np.zeros((NCORES, NPAD), np.int64)
    for c in range(NCORES):
        wd[c, :NPC] = wdeg[c * NPC:c * NPC + NPC]
    W2 = wd.reshape(NCORES, BLOCKS, 128).max(axis=(0, 2))
    W2 = np.maximum(W2 + (W2 % 2), 2)            # even, >=2
    starts = np.r_[0, np.cumsum(W2 * 128)]
    TOTS = int(starts[-1])

    coreP = dstP // NPC
    nlP = dstP - coreP * NPC
    blockP = nlP // 128
    vP = nlP % 128
    slotP = starts[blockP] + rankP * 128 + vP

    lov = np.where(srcP < HALF, srcP, HALF).astype(np.int16)
    hiv = np.where(srcP >= HALF, srcP - HALF, HALF).astype(np.int16)

    # chunks of whole blocks, sum W2 <= CHCAP, each >= table for efficiency
    chunks = []
    b0 = 0
    while b0 < BLOCKS:
        b1, tot = b0, 0
        while b1 < BLOCKS and tot + W2[b1] <= CHCAP:
            tot += W2[b1]
            b1 += 1
        chunks.append((b0, b1, int(starts[b0]), int(tot * 128)))
        b0 = b1
    CH = tuple(chunks)

    pidx_pc = []
    for c in range(NCORES):
        plo = np.full(TOTS, HALF, np.int16)
        phi = np.full(TOTS, HALF, np.int16)
        sel = coreP == c
        plo[slotP[sel]] = lov[sel]
        phi[slotP[sel]] = hiv[sel]
        pid = np.zeros((96, TOTS // 16), np.int16)
        for (cb0, cb1, s0, n) in CH:
            wlo = plo[s0:s0 + n].reshape(n // 16, 16).T
            whi = phi[s0:s0 + n].reshape(n // 16, 16).T
            c0 = s0 // 16
            for g in range(3):
                pid[g * 16:(g + 1) * 16, c0:c0 + n // 16] = wlo
                pid[48 + g * 16:48 + (g + 1) * 16, c0:c0 + n // 16] = whi
        pidx_pc.append(np.ascontiguousarray(pid))

    ptab = np.zeros((96, NE2), np.float32)
    ptab[0:48, 0:HALF] = feats[0:HALF].T
    ptab[48:96, 0:HALF] = feats[HALF:WINDOW].T
    pdeg = np.ones((96, NE2), np.int16)
    dg = np.maximum(out_deg, 1)
    pdeg[0:48, 0:HALF] = dg[0:HALF][None, :]
    pdeg[48:96, 0:HALF] = dg[HALF:WINDOW][None, :]

    # ---------------- DMA lane (src >= WINDOW) ---------------------------
    dsel = ~poolm
    srcd, dstd = src[dsel], dst[dsel]

    par = (srcd & 1).astype(np.int64)
    pair = (srcd >> 1).astype(np.int64)

    fpair = np.zeros((PAIRS, 96), np.float32)
    fpair[: N // 2, 0:48] = feats[0::2]
    fpair[: N // 2, 48:96] = feats[1::2]

    def pairwise_deg(deg_half):
        full = np.concatenate([deg_half, np.zeros(PAIRS - N // 2, np.int32)])
        return np.ascontiguousarray(full.reshape(PAIRS // 128, 128).T)

    degE_dev = pairwise_deg(out_deg[0::2])
    degO_dev = pairwise_deg(out_deg[1::2])

    key = dstd * 2 + par
    sort2 = np.argsort(key, kind="stable")
    ks = key[sort2]
    runstart = np.r_[0, np.flatnonzero(np.diff(ks)) + 1]
    runid = np.zeros(len(ks), np.int64)
    runid[runstart] = 1
    runid = np.cumsum(runid) - 1
    rank = np.empty(len(ks), np.int64)
    rank[sort2] = np.arange(len(ks)) - runstart[runid]

    core = dstd // NPC
    nl = dstd - core * NPC
    block = nl // 128
    v = nl % 128
    g = v // 32

    selm = rank < WMAIN
    Tm = par[selm] * WMAIN + g[selm] * (WMAIN // 4) + rank[selm] // 4
    lanem = (v[selm] % 32) * 4 + rank[selm] % 4

    selo = ~selm
    okey = (core[selo] * BLOCKS + block[selo]) * GROUPS + g[selo]
    osort = np.argsort(okey, kind="stable")
    oks = okey[osort]
    orunstart = np.r_[0, np.flatnonzero(np.diff(oks)) + 1]
    orunid = np.zeros(len(oks), np.int64)
    orunid[orunstart] = 1
    orunid = np.cumsum(orunid) - 1
    q = np.empty(len(oks), np.int64)
    q[osort] = np.arange(len(oks)) - orunstart[orunid]

    lvl1 = q < OVG_T * 128
    sel2 = ~lvl1
    oidx = np.flatnonzero(selo)
    e2 = oidx[sel2]
    k2 = core[e2] * BLOCKS + block[e2]
    s2 = np.argsort(k2, kind="stable")
    k2s = k2[s2]
    if len(k2s):
        rs2 = np.r_[0, np.flatnonzero(np.diff(k2s)) + 1]
        rid2 = np.zeros(len(k2s), np.int64)
        rid2[rs2] = 1
        rid2 = np.cumsum(rid2) - 1
        q2 = np.empty(len(k2s), np.int64)
        q2[s2] = np.arange(len(k2s)) - rs2[rid2]
        OV2_T = max(1, int(np.ceil((q2.max() + 1) / 128)))
    else:
        q2 = np.zeros(0, np.int64)
        OV2_T = 1
    assert OV2_T <= 4, f"unexpectedly deep level-2 overflow: {OV2_T}"

    TILES = 2 * WMAIN + GROUPS * OVG_T + OV2_T
    NIDX = TILES * 128

    gidx = np.full((NCORES, BLOCKS, TILES, 128), ZPAIR, np.int32)
    mg = np.zeros((NCORES, BLOCKS, GROUPS * OVG_T, 2, 128, 32), np.float16)
    m2 = np.zeros((NCORES, BLOCKS, OV2_T, 2, 128, 128), np.float16)

    gidx[core[selm], block[selm], Tm, lanem] = pair[selm]

    e1 = oidx[lvl1]
    t1 = q[lvl1] // 128
    lane1 = q[lvl1] % 128
    T1 = 2 * WMAIN + g[e1] * OVG_T + t1
    gidx[core[e1], block[e1], T1, lane1] = pair[e1]
    mg[core[e1], block[e1], g[e1] * OVG_T + t1, par[e1], lane1, v[e1] % 32] = 1.0

    t2 = q2 // 128
    lane2 = q2 % 128
    T2 = 2 * WMAIN + GROUPS * OVG_T + t2
    gidx[core[e2], block[e2], T2, lane2] = pair[e2]
    m2[core[e2], block[e2], t2, par[e2], lane2, v[e2]] = 1.0

    base_cnt = (2 * WMAIN + GROUPS * OVG_T) * 128
    ov2_cnt = np.zeros((NCORES, BLOCKS), np.int64)
    np.add.at(ov2_cnt, (core[e2], block[e2]), 1)
    counts = base_cnt + np.where(np.arange(BLOCKS)[None, :] < 5,
                                 OV2_T * 128, ov2_cnt)
    ar = np.arange(128)
    for t in range(OV2_T):
        pad = (ar[None, None, :] >=
               np.clip(ov2_cnt - t * 128, 0, 128)[:, :, None])
        pad = pad & (np.arange(BLOCKS)[None, :, None] >= 5)
        gidx[:, :, 2 * WMAIN + GROUPS * OVG_T + t, :][pad] = -1

    per_core = []
    for c in range(NCORES):
        flat = gidx[c].reshape(BLOCKS, NIDX).astype(np.int16)
        wrapped = flat.reshape(BLOCKS, NIDX // 16, 16).transpose(0, 2, 1)
        gidx_w = np.broadcast_to(
            wrapped[:, None, :, :], (BLOCKS, 8, 16, NIDX // 16)
        ).reshape(BLOCKS, 128, NIDX // 16).copy()

        mg_dev = np.ascontiguousarray(
            mg[c].transpose(0, 3, 1, 2, 4).reshape(
                BLOCKS, 128, GROUPS * OVG_T * 2 * 32))
        m2_dev = np.ascontiguousarray(
            m2[c].transpose(0, 3, 1, 2, 4).reshape(
                BLOCKS, 128, OV2_T * 2 * 128))

        nlo = c * NPC
        ind = np.zeros(NPAD, np.int32)
        ind[:NPC] = in_deg[nlo:nlo + NPC]
        indeg_dev = np.ascontiguousarray(ind.reshape(BLOCKS, 128).T)

        fc = np.zeros((NPAD, D), np.float32)
        fc[:NPC] = feats[nlo:nlo + NPC]
        featc_dev = np.ascontiguousarray(
            fc.reshape(BLOCKS, 128, D).transpose(1, 0, 2))

        blk = np.concatenate([
            gidx_w.view(np.uint8).reshape(BLOCKS, 128, -1),
            mg_dev.view(np.uint8).reshape(BLOCKS, 128, -1),
            m2_dev.view(np.uint8).reshape(BLOCKS, 128, -1),
        ], axis=2)
        per_core.append(dict(blk=np.ascontiguousarray(blk),
                             counts=np.ascontiguousarray(
                                 counts[c].astype(np.int32)[None, :]),
                             indeg=indeg_dev, featc=featc_dev,
                             pidx=pidx_pc[c]))

    meta = dict(OV2_T=OV2_T, TILES=TILES, NIDX=NIDX,
                W2=tuple(int(x) for x in W2), CH=CH, TOTS=TOTS)
    return fpair, degE_dev, degO_dev, ptab, pdeg, per_core, meta


def _build_program(meta):
    import concourse.tile as tile
    from concourse import bacc, mybir

    OV2_T, TILES, NIDX = meta["OV2_T"], meta["TILES"], meta["NIDX"]
    W2, CH, TOTS = meta["W2"], meta["CH"], meta["TOTS"]
    starts = [0]
    for w in W2:
        starts.append(starts[-1] + w * 128)
    f16 = mybir.dt.float16
    bf16 = mybir.dt.bfloat16
    f32 = mybir.dt.float32
    i32 = mybir.dt.int32
    i16 = mybir.dt.int16
    AF = mybir.ActivationFunctionType
    OP = mybir.AluOpType

    nc = bacc.Bacc("TRN2", target_bir_lowering=False, debug=False,
                   num_devices=NCORES, num_swdge_queues=2)

    fpair = nc.dram_tensor("fpair", [PAIRS, 96], f32, kind="ExternalInput").ap()
    degE = nc.dram_tensor("degE", [128, PAIRS // 128], i32, kind="ExternalInput").ap()
    degO = nc.dram_tensor("degO", [128, PAIRS // 128], i32, kind="ExternalInput").ap()
    GIB = (NIDX // 16) * 2
    MGB = GROUPS * OVG_T * 2 * 32 * 2
    M2B = OV2_T * 2 * 128 * 2
    BLKB = GIB + MGB + M2B
    u8 = mybir.dt.uint8
    blkD = nc.dram_tensor("blk", [BLOCKS, 128, BLKB], u8, kind="ExternalInput").ap()
    cntD = nc.dram_tensor("cnt", [1, BLOCKS], i32, kind="ExternalInput").ap()
    indegD = nc.dram_tensor("indeg", [128, BLOCKS], i32, kind="ExternalInput").ap()
    featcD = nc.dram_tensor("featc", [128, BLOCKS, D], f32, kind="ExternalInput").ap()
    wbD = nc.dram_tensor("wb", [D + 1, D], f32, kind="ExternalInput").ap()
    identD = nc.dram_tensor("ident", [128, 128], f32, kind="ExternalInput").ap()
    id32D = nc.dram_tensor("id32", [128, GROUPS * 32], f16, kind="ExternalInput").ap()
    ptabD = nc.dram_tensor("ptab", [96, NE2], f32, kind="ExternalInput").ap()
    pdegD = nc.dram_tensor("pdeg", [96, NE2], i16, kind="ExternalInput").ap()
    pidxD = nc.dram_tensor("pidx", [96, TOTS // 16], i16, kind="ExternalInput").ap()
    foldD = nc.dram_tensor("foldm", [96, D], f32, kind="ExternalInput").ap()
    outD = nc.dram_tensor("out", [D, NPAD], f32, kind="ExternalOutput").ap()

    xtab = nc.dram_tensor("xtab", [PAIRS, 128], f16).ap()
    CP = PAIRS // 128  # 196 pair-columns

    with tile.TileContext(nc) as tc:
        with tc.tile_pool(name="const", bufs=1) as cpool, \
             tc.tile_pool(name="big", bufs=1) as bigpool:

            wb_s = cpool.tile([D + 1, D], f32, tag="wb")
            nc.sync.dma_start(out=wb_s[:], in_=wbD)
            wb_b = cpool.tile([D + 1, D], bf16, tag="wbb")
            nc.vector.tensor_copy(wb_b[:], wb_s[:])
            ident = cpool.tile([128, 128], f32, tag="ident")
            nc.sync.dma_start(out=ident[:], in_=identD)
            id32 = cpool.tile([128, GROUPS * 32], f16, tag="id32")
            nc.sync.dma_start(out=id32[:], in_=id32D)
            foldf = cpool.tile([96, D], f32, tag="foldf")
            nc.sync.dma_start(out=foldf[:], in_=foldD)
            foldb = cpool.tile([96, D], bf16, tag="foldb")
            nc.vector.tensor_copy(foldb[:], foldf[:])

            # ---- pool lane table: load + ci scale ------------------------
            ptab_s = bigpool.tile([96, NE2], f32, tag="ptab")
            nc.sync.dma_start(out=ptab_s[:], in_=ptabD)

            # ---- phase 1: ci per parity + fp16 scaled pair table ---------
            ciE = cpool.tile([128, CP], f32, tag="ciE")
            ciO = cpool.tile([128, CP], f32, tag="ciO")
            with tc.tile_pool(name="xb", bufs=2) as xbpool:
                for deg_ap, ci in ((degE, ciE), (degO, ciO)):
                    dint = xbpool.tile([128, CP], i32, tag="dint")
                    nc.sync.dma_start(out=dint[:], in_=deg_ap)
                    nc.vector.tensor_copy(ci[:], dint[:])
                    nc.vector.tensor_scalar_max(ci[:], ci[:], 1.0)
                    nc.scalar.activation(ci[:], ci[:], AF.Sqrt)
                    nc.vector.reciprocal(ci[:], ci[:])

                XC = 49
                for cc in range(CP // XC):
                    sl = slice(cc * XC, (cc + 1) * XC)
                    fin = xbpool.tile([128, XC, 96], f32, tag="fin")
                    nc.sync.dma_start(
                        out=fin[:],
                        in_=fpair.rearrange("(c p) d -> p c d", p=128)[:, sl, :])
                    xt = xbpool.tile([128, XC, 128], f16, tag="xt")
                    nc.vector.tensor_tensor(
                        xt[:, :, 0:48], fin[:, :, 0:48],
                        ciE[:, sl].unsqueeze(2).to_broadcast([128, XC, 48]),
                        OP.mult)
                    nc.vector.tensor_tensor(
                        xt[:, :, 64:112], fin[:, :, 48:96],
                        ciO[:, sl].unsqueeze(2).to_broadcast([128, XC, 48]),
                        OP.mult)
                    nc.sync.dma_start(
                        out=xtab.rearrange("(c p) d -> p c d", p=128)[:, sl, :],
                        in_=xt[:])

            # pool table scaling: ptab *= rsqrt(clamped outdeg)
            with tc.tile_pool(name="cif", bufs=1) as cifpool:
                cif = cifpool.tile([96, NE2], f32, tag="cif")
                pdg = cifpool.tile([96, NE2], i16, tag="pdg")
                nc.sync.dma_start(out=pdg[:], in_=pdegD)
                nc.vector.tensor_copy(cif[:], pdg[:])
                nc.scalar.activation(cif[:], cif[:], AF.Sqrt)
                nc.vector.reciprocal(cif[:], cif[:])
                nc.vector.tensor_mul(ptab_s[:], ptab_s[:], cif[:])

            # ---- per-node scaling/blend coefficients ---------------------
            indeg_f = cpool.tile([128, BLOCKS], f32, tag="indegf")
            with tc.tile_pool(name="tmp0", bufs=1) as t0pool:
                indeg_i = t0pool.tile([128, BLOCKS], i32, tag="indegi")
                nc.sync.dma_start(out=indeg_i[:], in_=indegD)
                nc.vector.tensor_copy(indeg_f[:], indeg_i[:])
            mask = cpool.tile([128, BLOCKS], f32, tag="mask")
            nc.vector.tensor_scalar(mask[:], indeg_f[:], 0.0, None, OP.is_gt)
            cjm = cpool.tile([128, BLOCKS], f32, tag="cjm")
            nc.vector.tensor_scalar_max(cjm[:], indeg_f[:], 1.0)
            nc.scalar.activation(cjm[:], cjm[:], AF.Sqrt)
            nc.vector.reciprocal(cjm[:], cjm[:])
            nc.vector.tensor_mul(cjm[:], cjm[:], mask[:])
            im1 = cpool.tile([128, BLOCKS], f32, tag="im1")
            nc.vector.tensor_scalar(im1[:], mask[:], -1.0, 1.0,
                                    OP.mult, OP.add)

            featc_s = bigpool.tile([128, BLOCKS, D], f32, tag="featc")
            nc.sync.dma_start(out=featc_s[:], in_=featcD)
            fb_s = bigpool.tile([128, BLOCKS, D], f32, tag="fb")
            nc.vector.tensor_tensor(
                fb_s[:], featc_s[:],
                im1[:, :].unsqueeze(2).to_broadcast([128, BLOCKS, D]),
                OP.mult)
            hT1 = bigpool.tile([D + 1, BLOCKS * 128], bf16, tag="hT1")
            nc.vector.memset(hT1[:, :], 1.0)

            # ---- phase 2 ------------------------------------------------
            cnt_s = cpool.tile([1, BLOCKS], i32, tag="cnt")
            nc.sync.dma_start(out=cnt_s[:], in_=cntD)
            nidx_reg_a = nc.gpsimd.alloc_register("nidx_a")
            nidx_reg_b = nc.gpsimd.alloc_register("nidx_b")
            nidx_regs = [nidx_reg_a, nidx_reg_b]
            with tc.tile_pool(name="blk", bufs=3) as blkpool, \
                 tc.tile_pool(name="msg", bufs=3) as msgpool, \
                 tc.tile_pool(name="pox", bufs=1) as poxpool, \
                 tc.tile_pool(name="pix", bufs=2) as pixpool, \
                 tc.tile_pool(name="pob", bufs=2) as pobpool, \
                 tc.tile_pool(name="sm", bufs=4) as smpool, \
                 tc.tile_pool(name="ps", bufs=2, space="PSUM") as pspool, \
                 tc.tile_pool(name="pp", bufs=2, space="PSUM") as pppool, \
                 tc.tile_pool(name="aux", bufs=1, space="PSUM") as auxpool:

                pout = poxpool.tile([96, CHCAP * 128], f32, tag="pout")

                for (cb0, cb1, s0, nch) in CH:
                    # issue DMA-lane desc-gen for the chunk's blocks first so
                    # SDMA transfers overlap the ap_gather on Pool
                    blkts, msgs = {}, {}
                    for b in range(cb0, cb1):
                        blkt = blkpool.tile([128, BLKB], u8, tag="blkt")
                        nc.sync.dma_start(out=blkt[:], in_=blkD[b])
                        gi = blkt[:, 0:GIB].bitcast(i16)
                        msg = msgpool.tile([128, TILES, 128], f16, tag="msg")
                        nc.gpsimd.reg_load(nidx_regs[b % 2], cnt_s[0:1, b:b + 1])
                        nc.gpsimd.dma_gather(
                            out_ap=msg[:],
                            in_ap=xtab,
                            idxs_ap=gi,
                            num_idxs=NIDX,
                            num_idxs_reg=nidx_regs[b % 2],
                            elem_size=128,
                            queue_num=b % 2,
                            single_packet=False,
                        )
                        blkts[b], msgs[b] = blkt, msg

                    pidxt = pixpool.tile([96, CHCAP * 8], i16, tag="pidxt")
                    nc.sync.dma_start(
                        out=pidxt[:, 0:nch // 16],
                        in_=pidxD[:, s0 // 16:(s0 + nch) // 16])
                    nc.gpsimd.ap_gather(
                        out_ap=pout[:, 0:nch].unsqueeze(2),
                        in_ap=ptab_s[:].unsqueeze(2),
                        idxs_ap=pidxt[:, 0:nch // 16],
                        channels=96, num_elems=NE2, d=1, num_idxs=nch)

                    for b in range(cb0, cb1):
                        w2 = W2[b]
                        loff = starts[b] - s0
                        nb = w2 * 128
                        blkt, msg = blkts[b], msgs[b]
                        mgt = blkt[:, GIB:GIB + MGB].bitcast(f16)
                        m2t = blkt[:, GIB + MGB:BLKB].bitcast(f16)

                        # pool partial: bf16 convert + fold-mm accumulate
                        poutb = pobpool.tile([96, CHCAP * 128 // 3], bf16,
                                             tag="poutb")
                        if b % 2 == 0:
                            nc.scalar.activation(poutb[:, 0:nb],
                                                 pout[:, loff:loff + nb],
                                                 AF.Identity)
                        else:
                            nc.vector.tensor_copy(poutb[:, 0:nb],
                                                  pout[:, loff:loff + nb])
                        ps2 = pppool.tile([48, 256], f32, tag="ps2")
                        nmm = nb // 256
                        for j in range(nmm):
                            nc.tensor.matmul(
                                ps2[:, :], lhsT=foldb[:],
                                rhs=poutb[:, j * 256:(j + 1) * 256],
                                start=(j == 0), stop=(j == nmm - 1),
                                skip_group_check=True)
                        s2 = smpool.tile([48, 256], f32, tag="s2")
                        nc.vector.tensor_copy(s2[:], ps2[:])
                        pagg = smpool.tile([48, 128], f32, tag="pagg")
                        nc.vector.tensor_add(pagg[:], s2[:, 0:128],
                                             s2[:, 128:256])
                        ps3 = pspool.tile([128, 48], f32, tag="tp2")
                        nc.tensor.transpose(ps3[:], pagg[:],
                                            ident[0:48, 0:48])
                        pu = smpool.tile([128, 48], f32, tag="pu")
                        nc.vector.tensor_copy(pu[:], ps3[:])

                        ps = pspool.tile([128, D], f32, tag="ps")
                        JW = WMAIN // 4
                        for p, c0 in ((0, 0), (1, 64)):
                            for gg in range(GROUPS):
                                for j in range(JW):
                                    T = p * WMAIN + gg * JW + j
                                    nc.tensor.matmul(
                                        ps[32 * gg:32 * (gg + 1), :],
                                        lhsT=id32[:, 32 * gg:32 * (gg + 1)],
                                        rhs=msg[:, T, c0:c0 + D],
                                        start=(p == 0 and j == 0),
                                        stop=False, skip_group_check=True,
                                        tile_position=(0, 32 * gg))
                        for gg in range(GROUPS):
                            for t in range(OVG_T):
                                T = 2 * WMAIN + gg * OVG_T + t
                                base = (gg * OVG_T + t) * 64
                                for p, c0 in ((0, 0), (1, 64)):
                                    nc.tensor.matmul(
                                        ps[32 * gg:32 * (gg + 1), :],
                                        lhsT=mgt[:, base + p * 32:base + p * 32 + 32],
                                        rhs=msg[:, T, c0:c0 + D],
                                        start=False, stop=False,
                                        skip_group_check=True,
                                        tile_position=(0, 32 * gg))
                        for t in range(OV2_T):
                            T = 2 * WMAIN + GROUPS * OVG_T + t
                            base = t * 256
                            for pi, (p, c0) in enumerate(((0, 0), (1, 64))):
                                last = (t == OV2_T - 1) and (pi == 1)
                                nc.tensor.matmul(
                                    ps[:, :],
                                    lhsT=m2t[:, base + p * 128:base + (p + 1) * 128],
                                    rhs=msg[:, T, c0:c0 + D],
                                    start=False, stop=last,
                                    skip_group_check=True,
                                    tile_position=(0, 0))

                        # combine lanes, cj-scale, blend, transpose
                        sagg = smpool.tile([128, D], f32, tag="sagg")
                        nc.vector.tensor_add(sagg[:], ps[:], pu[:])
                        t0 = smpool.tile([128, D], f32, tag="t0")
                        nc.vector.tensor_tensor(
                            t0[:], sagg[:],
                            cjm[:, b:b + 1].to_broadcast([128, D]), OP.mult)
                        hb = smpool.tile([128, D], f32, tag="hb")
                        nc.vector.tensor_add(hb[:], t0[:], fb_s[:, b, :])

                        tp = auxpool.tile([D, 128], f32, tag="aux")
                        nc.tensor.transpose(tp[:], hb[:], ident[:])
                        nc.vector.tensor_copy(hT1[0:D, b * 128:(b + 1) * 128],
                                              tp[:])

                # ---- linear + relu (transposed) --------------------------
                CHN = 512
                outT = bigpool.tile([D, BLOCKS * 128], f32, tag="outT")
                nch2 = (BLOCKS * 128 + CHN - 1) // CHN
                for i in range(nch2):
                    lo = i * CHN
                    hi = min(lo + CHN, BLOCKS * 128)
                    po = auxpool.tile([D, CHN], f32, tag="po")
                    nc.tensor.matmul(po[:, 0:hi - lo], lhsT=wb_b[:],
                                     rhs=hT1[:, lo:hi], start=True, stop=True)
                    nc.scalar.activation(outT[:, lo:hi], po[:, 0:hi - lo],
                                         AF.Relu)

                nc.sync.dma_start(out=outD, in_=outT[:])

    nc.compile()
    return nc


def kernel(features, src, dst, W, b):
    from concourse.bass_utils import run_bass_kernel_spmd

    fpair, degE, degO, ptab, pdeg, per_core, meta = _host_prep(
        features, src, dst)

    key = (meta["OV2_T"], meta["W2"], meta["CH"])
    if key not in _CACHE:
        _CACHE[key] = _build_program(meta)
    nc = _CACHE[key]

    Wb = np.concatenate([np.asarray(W, np.float32).T,
                         np.asarray(b, np.float32)[None, :]], axis=0)
    ident = np.eye(128, dtype=np.float32)
    id32 = np.zeros((128, GROUPS * 32), np.float16)
    lanes = np.arange(128)
    for gg in range(GROUPS):
        id32[lanes, gg * 32 + lanes // 4] = 1.0
    foldm = np.zeros((96, D), np.float32)
    for c in range(D):
        foldm[c, c] = 1.0
        foldm[48 + c, c] = 1.0

    in_maps = []
    for c in range(NCORES):
        pc = per_core[c]
        in_maps.append({
            "fpair": fpair, "degE": degE, "degO": degO,
            "blk": pc["blk"], "cnt": pc["counts"],
            "indeg": pc["indeg"], "featc": pc["featc"],
            "wb": Wb, "ident": ident, "id32": id32,
            "ptab": ptab, "pdeg": pdeg, "pidx": pc["pidx"],
            "foldm": foldm,
        })

    res = run_bass_kernel_spmd(nc, in_maps, core_ids=list(range(NCORES)))
    globals()["LAST_RESULTS"] = res
    out = np.concatenate(
        [res.results[c]["out"][:, :NPC].T for c in range(NCORES)], axis=0)
    return np.ascontiguousarray(out, dtype=np.float32)
